# revision 19
# baseline (speedup 1.0000x reference)
"""LocalSelfAttention (block-diagonal, block=50) Bass kernel for 8 trn2 cores.

Sharding: sequence-parallel over the 41 attention blocks (padded to 48 =
8 cores x 6 blocks). Each core computes QKV projections, block-local
multi-head attention, and the output projection for its 6 blocks (300
tokens, laid out as 6 x 64 partition-padded rows = 384). No collectives;
the host slices inputs per core and concatenates the per-core outputs.

All matmul data is fp16 (measured ~3e-4 per-matmul relative error on HW;
PSUM accumulation is fp32). Softmax runs without max-subtraction (logits
are O(+-6) for this problem family), with exp's accum_out providing the
denominator for free.
"""

import os
import sys
from contextlib import ExitStack

sys.path.insert(0, "/opt/trn_rl_repo")

import numpy as np

import concourse.bass as bass  # noqa: F401  (registers engine types)
import concourse.mybir as mybir
import concourse.tile as tile
from concourse import bacc
from concourse.bass_utils import run_bass_kernel_spmd

# ---- problem constants (hardcoded; kernel.py must be self-contained) ----
T, H = 2048, 2048
HEADS, DH = 16, 128
KBLK = 50          # attention block size (tokens)
NEG = -1e9
NCORES = 8
P = T + (KBLK - T % KBLK)          # padded seq len = 2050
NB = P // KBLK                     # 41 real blocks
NB_CORE = 6                        # blocks per core (8*6 = 48 >= 41)
BPAD = 64                          # per-block padded rows (64-aligned for matmul bases)
TPAD = NB_CORE * BPAD              # 384 padded tokens per core
SCALE = DH ** -0.5
NH_T = H // 128                    # 16 h-tiles of 128
F16 = mybir.dt.float16
F32 = mybir.dt.float32

_CACHE = {}


def _build_program():
    nc = bacc.Bacc("TRN2", target_bir_lowering=False, debug=False,
                   num_devices=NCORES)

    xT_d = nc.dram_tensor("xT", [H, TPAD], F16, kind="ExternalInput").ap()
    wq_d = nc.dram_tensor("wq", [H, H], F16, kind="ExternalInput").ap()
    wk_d = nc.dram_tensor("wk", [H, H], F16, kind="ExternalInput").ap()
    wv_d = nc.dram_tensor("wv", [H, H], F16, kind="ExternalInput").ap()
    wo_d = nc.dram_tensor("wo", [H, H], F16, kind="ExternalInput").ap()
    bias_d = nc.dram_tensor("bias", [KBLK, NB_CORE * HEADS * KBLK], F32,
                            kind="ExternalInput").ap()
    ident_d = nc.dram_tensor("ident", [BPAD, BPAD], F16, kind="ExternalInput").ap()
    y_d = nc.dram_tensor("y", [TPAD, H], F32, kind="ExternalOutput").ap()

    with tile.TileContext(nc) as tc, ExitStack() as ctx:
        _emit_body(nc, tc, ctx, xT_d, wq_d, wk_d, wv_d, wo_d, bias_d, ident_d, y_d)

    nc.compile()
    return nc


def _emit_body(nc, tc, ctx, xT_d, wq_d, wk_d, wv_d, wo_d, bias_d, ident_d, y_d):
    if True:
        sb = ctx.enter_context(tc.tile_pool(name="persist", bufs=1))
        wpool = ctx.enter_context(tc.tile_pool(name="wpool", bufs=32))
        apool = ctx.enter_context(tc.tile_pool(name="apool", bufs=3))
        ps_proj = ctx.enter_context(tc.tile_pool(name="ps_proj", bufs=2, space="PSUM"))
        ps_s = ctx.enter_context(tc.tile_pool(name="ps_s", bufs=2, space="PSUM"))
        ps_at = ctx.enter_context(tc.tile_pool(name="ps_at", bufs=2, space="PSUM"))
        ps_ot = ctx.enter_context(tc.tile_pool(name="ps_ot", bufs=2, space="PSUM"))

        # ---- persistent SBUF arrays ----
        xT = [sb.tile([128, TPAD], F16, tag=f"xT{h}", name=f"xT{h}") for h in range(NH_T)]
        qt = [sb.tile([128, TPAD], F16, tag=f"qt{e}", name=f"qt{e}") for e in range(HEADS)]
        kt = [sb.tile([128, TPAD], F16, tag=f"kt{e}", name=f"kt{e}") for e in range(HEADS)]
        ot = [sb.tile([128, TPAD], F16, tag=f"ot{e}", name=f"ot{e}") for e in range(HEADS)]
        vsb = [sb.tile([128, H], F16, tag=f"v{t}", name=f"vsb{t}") for t in range(3)]
        atb = [sb.tile([128, HEADS * KBLK], F16, tag=f"at{b}", name=f"atb{b}") for b in range(NB_CORE)]
        bias_sb = sb.tile([KBLK, NB_CORE * HEADS * KBLK], F32, tag="bias")
        ident = sb.tile([BPAD, BPAD], F16, tag="ident")

        for h in range(NH_T):
            nc.sync.dma_start(xT[h][:], xT_d[128 * h:128 * (h + 1), :])
        nc.sync.dma_start(bias_sb[:], bias_d)
        nc.sync.dma_start(ident[:], ident_d)

        phase = int(os.environ.get("BISECT_PHASE", "9"))

        def _dump(tiles):
            # bisect helper: route intermediate tiles to y
            for i, t_ in enumerate(tiles[:3]):
                w = min(512, t_.shape[-1])
                p = t_.shape[0]
                yt = apool.tile([128, 512], F32, tag="y", name="ydump")
                nc.vector.tensor_copy(yt[0:p, 0:w], t_[:, 0:w])
                nc.sync.dma_start(y_d[128 * i:128 * i + p, 0:w], yt[0:p, 0:w])

        # ---- QT / KT projections: out[ed, t] = W[h, ed].T @ xT[h, t] ----
        for w_hbm, dst in ((wq_d, qt), (wk_d, kt)):
            for eg in range(4):            # groups of 4 heads (512 ed cols)
                wt = [wpool.tile([128, 512], F16, tag="w", name="w") for _ in range(NH_T)]
                for h in range(NH_T):
                    nc.sync.dma_start(
                        wt[h][:], w_hbm[128 * h:128 * (h + 1), 512 * eg:512 * (eg + 1)])
                for el in range(4):        # head within group
                    e = 4 * eg + el
                    pt = ps_proj.tile([128, TPAD], F32, tag="proj")
                    for h in range(NH_T):
                        nc.tensor.matmul(pt[:], wt[h][:, 128 * el:128 * (el + 1)],
                                         xT[h][:], start=(h == 0), stop=(h == NH_T - 1))
                    nc.vector.tensor_copy(dst[e][:], pt[:])

        if phase <= 1:
            _dump(qt)
            return
        # ---- V projection, token-major: out[t, ed] = xT[h, t].T @ W[h, ed] ----
        for eg in range(4):
            wt = [wpool.tile([128, 512], F16, tag="w", name="w") for _ in range(NH_T)]
            for h in range(NH_T):
                nc.sync.dma_start(
                    wt[h][:], wv_d[128 * h:128 * (h + 1), 512 * eg:512 * (eg + 1)])
            for tt in range(3):            # token tiles of 128
                pt = ps_proj.tile([128, 512], F32, tag="proj")
                for h in range(NH_T):
                    nc.tensor.matmul(pt[:], xT[h][:, 128 * tt:128 * (tt + 1)],
                                     wt[h][:], start=(h == 0), stop=(h == NH_T - 1))
                nc.scalar.copy(vsb[tt][:, 512 * eg:512 * (eg + 1)], pt[:])

        if phase <= 2:
            _dump(vsb)
            return
        # ---- attention per block ----
        for b in range(NB_CORE):
            base = BPAD * (b % 2)          # 0 or 64: V/AT partition base for this block
            tcol = BPAD * b                # token column offset of this block
            asb = apool.tile([KBLK, HEADS * BPAD], F16, tag="a_exp")
            anrm = apool.tile([KBLK, HEADS * BPAD], F16, tag="a_nrm")
            # transposes read [50, 114] spans incl. the 50..63 gap columns
            nc.vector.memset(anrm[:], 0.0)
            sums = apool.tile([KBLK, HEADS], F32, tag="sums")
            recip = apool.tile([KBLK, HEADS], F32, tag="recip")
            for g in range(2):             # head groups of 8 (one PSUM bank each)
                sp = ps_s.tile([KBLK, 8 * KBLK], F32, tag="s")
                for el in range(8):
                    e = 8 * g + el
                    nc.tensor.matmul(sp[:, KBLK * el:KBLK * (el + 1)],
                                     qt[e][:, tcol:tcol + KBLK],
                                     kt[e][:, tcol:tcol + KBLK],
                                     start=True, stop=True)
                # logits = S*scale + (blockbias + maskbias); in-place in PSUM
                boff = (b * 2 + g) * 8 * KBLK
                nc.vector.scalar_tensor_tensor(
                    out=sp[:], in0=sp[:], scalar=SCALE,
                    in1=bias_sb[:, boff:boff + 8 * KBLK],
                    op0=mybir.AluOpType.mult, op1=mybir.AluOpType.add)
                for el in range(8):
                    e = 8 * g + el
                    nc.scalar.activation(
                        asb[:, BPAD * e:BPAD * e + KBLK],
                        sp[:, KBLK * el:KBLK * (el + 1)],
                        mybir.ActivationFunctionType.Exp,
                        accum_out=sums[:, e:e + 1])
            nc.vector.reciprocal(recip[:], sums[:])
            for e in range(HEADS):
                nc.vector.tensor_scalar_mul(
                    anrm[:, BPAD * e:BPAD * e + KBLK],
                    asb[:, BPAD * e:BPAD * e + KBLK], recip[:, e:e + 1])
            for j in range(HEADS // 2):
                # transpose head pairs [50, 114] -> [114, 50]: head 2j+1 sits at
                # free offset 64 so its k-rows land at partition base 64 (aligned)
                atp = ps_at.tile([128, KBLK], F16, tag="atp")
                nc.tensor.transpose(
                    atp[0:2 * BPAD - 14, :],
                    anrm[:, 2 * BPAD * j:2 * BPAD * j + 2 * BPAD - 14],
                    ident[0:KBLK, 0:KBLK])
                for p in range(2):
                    e = 2 * j + p
                    nc.vector.tensor_copy(
                        atb[b][base:base + KBLK, KBLK * e:KBLK * (e + 1)],
                        atp[BPAD * p:BPAD * p + KBLK, :])

        if phase <= 3:
            _dump(atb)
            return
        # ---- A^T @ V -> OT[dh, t] per head ----
        # Matmuls with different lhsT partition bases (row groups 0 vs 64) run
        # concurrently in the PE array and MUST NOT share a PSUM bank: use one
        # PSUM tile per block-parity, then strided copies into ot[e].
        for e in range(HEADS):
            nc.vector.memset(ot[e][:], 0.0)  # gap columns never written below
        for e in range(HEADS):
            opa = ps_ot.tile([128, TPAD], F32, tag="otp", name="opa")
            opb = ps_ot.tile([128, TPAD], F32, tag="otp", name="opb")
            opp = (opa, opb)
            for b in range(NB_CORE):
                par = b % 2
                base = BPAD * par
                nc.tensor.matmul(
                    opp[par][:, 128 * (b // 2):128 * (b // 2) + KBLK],
                    vsb[b // 2][base:base + KBLK, 128 * e:128 * (e + 1)],
                    atb[b][base:base + KBLK, KBLK * e:KBLK * (e + 1)],
                    start=True, stop=True)
            for par in range(2):
                src = opp[par].rearrange("p (b x) -> p b x", b=3)[:, :, 0:KBLK]
                dst = ot[e].rearrange("p (b q x) -> p b q x", b=3, q=2)[:, :, par, 0:KBLK]
                nc.scalar.copy(dst, src)

        if phase <= 4 or phase in (35, 37):
            _dump(ot)
            return
        # ---- output projection: y[t, hout] = OT[ed, t].T @ Wo[ed, hout] ----
        for hg in range(4):
            wt = [wpool.tile([128, 512], F16, tag="w", name="w") for _ in range(HEADS)]
            for e in range(HEADS):
                nc.sync.dma_start(
                    wt[e][:], wo_d[128 * e:128 * (e + 1), 512 * hg:512 * (hg + 1)])
            for tt in range(3):
                pt = ps_proj.tile([128, 512], F32, tag="proj")
                for e in range(HEADS):
                    nc.tensor.matmul(pt[:], ot[e][:, 128 * tt:128 * (tt + 1)],
                                     wt[e][:], start=(e == 0), stop=(e == HEADS - 1))
                ysb = apool.tile([128, 512], F32, tag="y")
                nc.vector.tensor_copy(ysb[:], pt[:])
                nc.sync.dma_start(
                    y_d[128 * tt:128 * (tt + 1), 512 * hg:512 * (hg + 1)], ysb[:])


def _prep_inputs(x, mask, bias, Wq, Wk, Wv, Wo):
    """Slice/pad/transpose the full inputs into per-core input maps."""
    x = np.asarray(x, np.float32).reshape(T, H)
    mask = np.asarray(mask, np.float32).reshape(T)
    bias = np.asarray(bias, np.float32).reshape(HEADS, T, T)

    # padded x (tokens) and mask, as in the reference
    xp = np.zeros((P, H), np.float32)
    xp[:T] = x
    mb = np.full(P, NEG, np.float32)
    mb[:T] = mask

    # combined per-block additive bias: block-diag of attention bias + pair mask
    comb = np.zeros((NB_CORE * NCORES, HEADS, KBLK, KBLK), np.float32)
    for b in range(NB):
        lo, hi = KBLK * b, KBLK * (b + 1)
        blk = np.zeros((HEADS, KBLK, KBLK), np.float32)
        lim = min(hi, T) - lo
        if lim > 0:
            blk[:, :lim, :lim] = bias[:, lo:lo + lim, lo:lo + lim]
        pair = mb[lo:hi, None] * mb[None, lo:hi]
        blk += np.where(pair > 0, 0.0, NEG)[None]
        comb[b] = blk

    wq = np.ascontiguousarray(Wq.reshape(H, H), np.float32).astype(np.float16)
    wk = np.ascontiguousarray(Wk.reshape(H, H), np.float32).astype(np.float16)
    wv = np.ascontiguousarray(Wv.reshape(H, H), np.float32).astype(np.float16)
    wo = np.ascontiguousarray(Wo.reshape(H, H), np.float32).astype(np.float16)
    ident = np.eye(BPAD, dtype=np.float16)

    in_maps = []
    for c in range(NCORES):
        xc = np.zeros((TPAD, H), np.float32)
        for j in range(NB_CORE):
            b = NB_CORE * c + j
            if b < NB:
                xc[BPAD * j:BPAD * j + KBLK] = xp[KBLK * b:KBLK * (b + 1)]
        # bias layout: [q, (block, headgroup, head, k)]
        bc = comb[NB_CORE * c:NB_CORE * (c + 1)]          # [6, 16, 50, 50]
        bc = bc.transpose(2, 0, 1, 3).reshape(KBLK, NB_CORE * HEADS * KBLK)
        in_maps.append({
            "xT": np.ascontiguousarray(xc.T).astype(np.float16),
            "wq": wq, "wk": wk, "wv": wv, "wo": wo,
            "bias": np.ascontiguousarray(bc),
            "ident": ident,
        })
    return in_maps


def _gather(results):
    out = np.empty((T, H), np.float32)
    for c in range(NCORES):
        yc = results[c]["y"]
        for j in range(NB_CORE):
            b = NB_CORE * c + j
            if b >= NB:
                continue
            n = min(KBLK * (b + 1), T) - KBLK * b
            if n > 0:
                out[KBLK * b:KBLK * b + n] = yc[BPAD * j:BPAD * j + n]
    return out


def run(trace=False, **inputs):
    if "nc" not in _CACHE:
        _CACHE["nc"] = _build_program()
    nc = _CACHE["nc"]
    in_maps = _prep_inputs(
        inputs["x_BxTxH"], inputs["mask_BxT"], inputs["attention_bias_BxHxTxT"],
        inputs["Wq"], inputs["Wk"], inputs["Wv"], inputs["Wo"])
    res = run_bass_kernel_spmd(nc, in_maps, list(range(NCORES)), trace=trace)
    out = _gather(res.results)[None]       # restore batch dim [1, T, H]
    return out, res.exec_time_ns


def kernel(**inputs):
    out, _ = run(trace=False, **inputs)
    return out


# revision 29
# speedup vs baseline: 1.3648x; 1.3648x over previous
"""LocalSelfAttention (block-diagonal, block=50) Bass kernel for 8 trn2 cores.

Sharding: sequence-parallel over the 41 attention blocks (padded to 48 =
8 cores x 6 blocks). Each core computes QKV projections, block-local
multi-head attention, and the output projection for its 6 blocks (300
tokens, laid out as 6 x 64 partition-padded rows = 384). No collectives;
the host slices inputs per core and concatenates the per-core outputs.

All matmul data is fp16 (measured ~3e-4 per-matmul relative error on HW;
PSUM accumulation is fp32). Softmax runs without max-subtraction (logits
are O(+-6) for this problem family), with exp's accum_out providing the
denominator for free.

Scheduling notes (trn2):
- Engines execute their instruction streams in emission order, so the
  attention/softmax chain for head-group g is emitted right after the
  QT/KT projections of that head-group: its DVE/ACT work then overlaps
  the remaining projection matmuls on the PE.
- Matmuls whose lhsT partition bases differ (row groups 0 vs 64) can run
  concurrently inside the PE array and must not target the same PSUM
  bank (hardware collision) - AV outputs are split by block parity.
- Weight streaming uses [128, 1024] fp16 chunks alternating between the
  two HWDGE queues (sync + scalar) to keep DMA off the critical path.
"""

import sys
from contextlib import ExitStack

sys.path.insert(0, "/opt/trn_rl_repo")

import numpy as np

import concourse.bass as bass  # noqa: F401
import concourse.mybir as mybir
import concourse.tile as tile
from concourse import bacc
from concourse.bass_utils import run_bass_kernel_spmd

# ---- problem constants (hardcoded; kernel.py must be self-contained) ----
T, H = 2048, 2048
HEADS, DH = 16, 128
KBLK = 50          # attention block size (tokens)
NEG = -1e9
NCORES = 8
P = T + (KBLK - T % KBLK)          # padded seq len = 2050
NB = P // KBLK                     # 41 real blocks
NB_CORE = 6                        # blocks per core (8*6 = 48 >= 41)
BPAD = 64                          # per-block padded rows (64-aligned matmul bases)
TPAD = NB_CORE * BPAD              # 384 padded tokens per core
SCALE = DH ** -0.5
NH_T = H // 128                    # 16 h-tiles of 128
TPACK = NB_CORE * KBLK             # 300 packed tokens (QT/KT only)
F16 = mybir.dt.float16
F32 = mybir.dt.float32

_CACHE = {}


def _build_program():
    nc = bacc.Bacc("TRN2", target_bir_lowering=False, debug=False,
                   num_devices=NCORES)

    xT_d = nc.dram_tensor("xT", [H, TPAD], F16, kind="ExternalInput").ap()
    xTq_d = nc.dram_tensor("xTq", [H, TPACK], F16, kind="ExternalInput").ap()
    wq_d = nc.dram_tensor("wq", [H, H], F16, kind="ExternalInput").ap()
    wk_d = nc.dram_tensor("wk", [H, H], F16, kind="ExternalInput").ap()
    wv_d = nc.dram_tensor("wv", [H, H], F16, kind="ExternalInput").ap()
    wo_d = nc.dram_tensor("wo", [H, H], F16, kind="ExternalInput").ap()
    bias_d = nc.dram_tensor("bias", [KBLK, NB_CORE * HEADS * KBLK], F16,
                            kind="ExternalInput").ap()
    ident_d = nc.dram_tensor("ident", [128, 128], F16, kind="ExternalInput").ap()
    y_d = nc.dram_tensor("y", [TPAD, H], F32, kind="ExternalOutput").ap()

    with tile.TileContext(nc) as tc, ExitStack() as ctx:
        _emit_body(nc, tc, ctx, xT_d, xTq_d, wq_d, wk_d, wv_d, wo_d, bias_d,
                   ident_d, y_d)

    nc.compile()
    return nc


def _emit_body(nc, tc, ctx, xT_d, xTq_d, wq_d, wk_d, wv_d, wo_d, bias_d,
               ident_d, y_d):
    sb = ctx.enter_context(tc.tile_pool(name="persist", bufs=1))
    wpool = ctx.enter_context(tc.tile_pool(name="wpool", bufs=32))
    apool = ctx.enter_context(tc.tile_pool(name="apool", bufs=3))
    anpool = ctx.enter_context(tc.tile_pool(name="anpool", bufs=7))
    ps_proj = ctx.enter_context(tc.tile_pool(name="ps_proj", bufs=3, space="PSUM"))
    ps_s = ctx.enter_context(tc.tile_pool(name="ps_s", bufs=2, space="PSUM"))
    ps_at = ctx.enter_context(tc.tile_pool(name="ps_at", bufs=1, space="PSUM"))
    ps_ot = ctx.enter_context(tc.tile_pool(name="ps_ot", bufs=2, space="PSUM"))

    # ---- persistent SBUF arrays ----
    xT = [sb.tile([128, TPAD], F16, tag=f"xT{h}", name=f"xT{h}") for h in range(NH_T)]
    xTq = [sb.tile([128, TPACK], F16, tag=f"xTq{h}", name=f"xTq{h}")
           for h in range(NH_T)]
    qt = [sb.tile([128, TPACK], F16, tag=f"qt{e}", name=f"qt{e}") for e in range(HEADS)]
    kt = [sb.tile([128, TPACK], F16, tag=f"kt{e}", name=f"kt{e}") for e in range(HEADS)]
    ot = [sb.tile([128, TPAD], F16, tag=f"ot{e}", name=f"ot{e}") for e in range(HEADS)]
    vsb = [sb.tile([128, H], F16, tag=f"v{t}", name=f"vsb{t}") for t in range(3)]
    atb = [sb.tile([128, HEADS * KBLK], F16, tag=f"at{b}", name=f"atb{b}")
           for b in range(NB_CORE)]
    bias_sb = sb.tile([KBLK, NB_CORE * HEADS * KBLK], F16, tag="bias")
    ident = sb.tile([128, 128], F16, tag="ident")

    def wload(dst, src, h):
        # alternate the two HWDGE queues so weight streaming isn't serialized
        (nc.sync if h % 2 == 0 else nc.scalar).dma_start(dst, src)

    for e in range(HEADS):
        nc.vector.memset(ot[e][:], 0.0)  # OT gap columns are never written

    anrm_live = {}

    def attention_softmax(g):
        # S matmuls + softmax for heads 8g..8g+8 of every block -> anrm tiles.
        # The additive bias is injected by an identity-matmul accumulation into
        # the S PSUM bank (start=True) BEFORE the S matmuls (start=False) - all
        # at row group 0, so no concurrent-row-group bank hazard. The softmax
        # scale rides on exp's free affine; bias was pre-scaled by sqrt(DH) on
        # the host.
        for b in range(NB_CORE):
            tcol = KBLK * b
            asb = apool.tile([KBLK, 8 * BPAD], F16, tag="a_exp", name="asb")
            anrm = anpool.tile([KBLK, 8 * BPAD], F16, tag="a_nrm", name="anrm")
            nc.vector.memset(anrm[:], 0.0)  # transpose reads [50,114] spans
            anrm_live[(b, g)] = anrm
            sums = apool.tile([KBLK, 8], F32, tag="sums", name="sums")
            recip = apool.tile([KBLK, 8], F32, tag="recip", name="recip")
            sp = ps_s.tile([KBLK, 8 * KBLK], F32, tag="s", name="sp")
            boff = (b * 2 + g) * 8 * KBLK
            nc.tensor.matmul(sp[:], ident[0:KBLK, 0:KBLK],
                             bias_sb[:, boff:boff + 8 * KBLK],
                             start=True, stop=False)
            for el in range(8):
                e = 8 * g + el
                nc.tensor.matmul(sp[:, KBLK * el:KBLK * (el + 1)],
                                 qt[e][:, tcol:tcol + KBLK],
                                 kt[e][:, tcol:tcol + KBLK],
                                 start=False, stop=(el == 7))
            nc.scalar.activation(
                asb.rearrange("p (e x) -> p e x", e=8)[:, :, 0:KBLK],
                sp[:], mybir.ActivationFunctionType.Exp, scale=SCALE)
            nc.vector.reduce_sum(
                sums[:], asb.rearrange("p (e x) -> p e x", e=8)[:, :, 0:KBLK],
                axis=mybir.AxisListType.X)
            nc.vector.reciprocal(recip[:], sums[:])
            for el in range(8):
                nc.vector.tensor_scalar_mul(
                    anrm[:, BPAD * el:BPAD * el + KBLK],
                    asb[:, BPAD * el:BPAD * el + KBLK], recip[:, el:el + 1])

    def attention_transpose(g):
        # emitted a phase later so the PE never stalls on the softmax chain
        for b in range(NB_CORE):
            base = BPAD * (b % 2)
            anrm = anrm_live.pop((b, g))
            for j in range(4):
                # head pair (2j, 2j+1) within the group: [50, 114] -> [114, 50];
                # head 2j+1 sits at free offset 64 so its k-rows land on
                # partition base 64 (32-aligned PSUM reads)
                atp = ps_at.tile([128, KBLK], F16, tag="atp", name="atp")
                nc.tensor.transpose(
                    atp[0:2 * BPAD - 14, :],
                    anrm[:, 2 * BPAD * j:2 * BPAD * (j + 1) - 14],
                    ident[0:KBLK, 0:KBLK])
                for p_ in range(2):
                    e = 8 * g + 2 * j + p_
                    nc.vector.tensor_copy(
                        atb[b][base:base + KBLK, KBLK * e:KBLK * (e + 1)],
                        atp[BPAD * p_:BPAD * p_ + KBLK, :])

    # ---- QT/KT projections (per half of the heads) + interleaved attention --
    for g in range(2):
        for w_hbm, dst in ((wq_d, qt), (wk_d, kt)):
            wt = [wpool.tile([128, 1024], F16, tag="w", name="w")
                  for _ in range(NH_T)]
            for h in range(NH_T):
                wload(wt[h][:],
                      w_hbm[128 * h:128 * (h + 1), 1024 * g:1024 * (g + 1)], h)
                if g == 0 and w_hbm is wq_d:
                    # first pass also stages packed x, paired h-by-h with the
                    # weight chunks so the first QT matmuls start immediately
                    wload(xTq[h][:], xTq_d[128 * h:128 * (h + 1), :], h + 1)
                if g == 0 and w_hbm is wk_d:
                    # padded x is first needed by the V phase - keep it off the
                    # QT critical path
                    wload(xT[h][:], xT_d[128 * h:128 * (h + 1), :], h + 1)
            if g == 0 and w_hbm is wk_d:
                # bias/ident aren't needed until the first attention ops
                # (~35us in); keep them off the startup critical path
                nc.sync.dma_start(bias_sb[:], bias_d)
                nc.scalar.dma_start(ident[:], ident_d)
            for el in range(8):            # head within group
                e = 8 * g + el
                pt = ps_proj.tile([128, TPACK], F32, tag="proj", name="pt")
                for h in range(NH_T):
                    nc.tensor.matmul(pt[:], wt[h][:, 128 * el:128 * (el + 1)],
                                     xTq[h][:], start=(h == 0), stop=(h == NH_T - 1))
                nc.vector.tensor_copy(dst[e][:], pt[:])
        attention_softmax(g)
        if g == 1:
            attention_transpose(0)

    # ---- V projection, token-major: out[t, ed] = xT[h, t].T @ W[h, ed] ----
    wt_wo0 = None
    for g in range(2):
        if g == 1:
            attention_transpose(1)
        wt = [wpool.tile([128, 1024], F16, tag="w", name="w") for _ in range(NH_T)]
        for h in range(NH_T):
            wload(wt[h][:], wv_d[128 * h:128 * (h + 1), 1024 * g:1024 * (g + 1)], h)
        if g == 1:
            # stage Wo group 0 now; its slot-waits resolve as V-g0 chunks
            # release, well before the output projection needs the data
            wt_wo0 = [wpool.tile([128, 1024], F16, tag="w", name="w")
                      for _ in range(HEADS)]
            for e in range(HEADS):
                wload(wt_wo0[e][:], wo_d[128 * e:128 * (e + 1), 0:1024], e + 1)
        for eo in range(2):
            eg = 2 * g + eo
            for tt in range(3):            # token tiles of 128
                pt = ps_proj.tile([128, 512], F32, tag="proj", name="pt")
                for h in range(NH_T):
                    nc.tensor.matmul(pt[:], xT[h][:, 128 * tt:128 * (tt + 1)],
                                     wt[h][:, 512 * eo:512 * (eo + 1)],
                                     start=(h == 0), stop=(h == NH_T - 1))
                nc.scalar.copy(vsb[tt][:, 512 * eg:512 * (eg + 1)], pt[:])

    # ---- A^T @ V -> OT[dh, t] per head ----
    # Matmuls with different lhsT partition bases (row groups 0 vs 64) run
    # concurrently in the PE array and must not share a PSUM bank: use one
    # PSUM tile per block-parity, then strided copies into ot[e].
    for e in range(HEADS):
        opa = ps_ot.tile([128, TPAD], F32, tag="otp", name="opa")
        opb = ps_ot.tile([128, TPAD], F32, tag="otp", name="opb")
        opp = (opa, opb)
        for b in range(NB_CORE):
            par = b % 2
            base = BPAD * par
            nc.tensor.matmul(
                opp[par][:, 128 * (b // 2):128 * (b // 2) + KBLK],
                vsb[b // 2][base:base + KBLK, 128 * e:128 * (e + 1)],
                atb[b][base:base + KBLK, KBLK * e:KBLK * (e + 1)],
                start=True, stop=True)
        for par in range(2):
            src = opp[par].rearrange("p (b x) -> p b x", b=3)[:, :, 0:KBLK]
            dst = ot[e].rearrange("p (b q x) -> p b q x", b=3, q=2)[:, :, par, 0:KBLK]
            nc.scalar.copy(dst, src)

    # ---- output projection: y[t, hout] = OT[ed, t].T @ Wo[ed, hout] ----
    for g in range(2):
        if g == 0:
            wt = wt_wo0
        else:
            wt = [wpool.tile([128, 1024], F16, tag="w", name="w")
                  for _ in range(HEADS)]
            for e in range(HEADS):
                wload(wt[e][:], wo_d[128 * e:128 * (e + 1), 1024:2048], e)
        for ho in range(2):
            hg = 2 * g + ho
            for tt in range(3):
                pt = ps_proj.tile([128, 512], F32, tag="proj", name="pt")
                for e in range(HEADS):
                    nc.tensor.matmul(pt[:], ot[e][:, 128 * tt:128 * (tt + 1)],
                                     wt[e][:, 512 * ho:512 * (ho + 1)],
                                     start=(e == 0), stop=(e == HEADS - 1))
                ysb = apool.tile([128, 512], F32, tag="y", name="ysb")
                nc.vector.tensor_copy(ysb[:], pt[:])
                nc.sync.dma_start(
                    y_d[128 * tt:128 * (tt + 1), 512 * hg:512 * (hg + 1)], ysb[:])


def _prep_inputs(x, mask, bias, Wq, Wk, Wv, Wo):
    """Slice/pad/transpose the full inputs into per-core input maps."""
    x = np.asarray(x, np.float32).reshape(T, H)
    mask = np.asarray(mask, np.float32).reshape(T)
    bias = np.asarray(bias, np.float32).reshape(HEADS, T, T)

    # padded x (tokens) and mask, as in the reference
    xp = np.zeros((P, H), np.float32)
    xp[:T] = x
    mb = np.full(P, NEG, np.float32)
    mb[:T] = mask

    # combined per-block additive bias: block-diag of attention bias + pair mask
    comb = np.zeros((NB_CORE * NCORES, HEADS, KBLK, KBLK), np.float32)
    for b in range(NB):
        lo, hi = KBLK * b, KBLK * (b + 1)
        blk = np.zeros((HEADS, KBLK, KBLK), np.float32)
        lim = min(hi, T) - lo
        if lim > 0:
            blk[:, :lim, :lim] = bias[:, lo:lo + lim, lo:lo + lim]
        pair = mb[lo:hi, None] * mb[None, lo:hi]
        # pre-scaled by sqrt(DH) (exp applies scale=DH**-0.5 to S + bias);
        # masked entries use -4000 (fp16-safe; * SCALE -> -354, exp -> 0)
        blk = blk * (DH ** 0.5) + np.where(pair > 0, 0.0, -4000.0)[None]
        comb[b] = blk

    wq = np.ascontiguousarray(Wq.reshape(H, H), np.float32).astype(np.float16)
    wk = np.ascontiguousarray(Wk.reshape(H, H), np.float32).astype(np.float16)
    wv = np.ascontiguousarray(Wv.reshape(H, H), np.float32).astype(np.float16)
    wo = np.ascontiguousarray(Wo.reshape(H, H), np.float32).astype(np.float16)
    ident = np.eye(128, dtype=np.float16)

    in_maps = []
    for c in range(NCORES):
        xc = np.zeros((TPAD, H), np.float32)
        for j in range(NB_CORE):
            b = NB_CORE * c + j
            if b < NB:
                xc[BPAD * j:BPAD * j + KBLK] = xp[KBLK * b:KBLK * (b + 1)]
        # bias layout: [q, (block, headgroup, head, k)], fp16
        cc = comb[NB_CORE * c:NB_CORE * (c + 1)]          # [6, 16, 50, 50]
        bc = cc.transpose(2, 0, 1, 3).reshape(KBLK, NB_CORE * HEADS * KBLK)
        bc = bc.astype(np.float16)
        xq = np.zeros((TPACK, H), np.float32)
        for j in range(NB_CORE):
            b = NB_CORE * c + j
            if b < NB:
                xq[KBLK * j:KBLK * (j + 1)] = xp[KBLK * b:KBLK * (b + 1)]
        in_maps.append({
            "xT": np.ascontiguousarray(xc.T).astype(np.float16),
            "xTq": np.ascontiguousarray(xq.T).astype(np.float16),
            "wq": wq, "wk": wk, "wv": wv, "wo": wo,
            "bias": np.ascontiguousarray(bc),
            "ident": ident,
        })
    return in_maps


def _gather(results):
    out = np.empty((T, H), np.float32)
    for c in range(NCORES):
        yc = results[c]["y"]
        for j in range(NB_CORE):
            b = NB_CORE * c + j
            if b >= NB:
                continue
            n = min(KBLK * (b + 1), T) - KBLK * b
            if n > 0:
                out[KBLK * b:KBLK * b + n] = yc[BPAD * j:BPAD * j + n]
    return out


def run(trace=False, **inputs):
    if "nc" not in _CACHE:
        _CACHE["nc"] = _build_program()
    nc = _CACHE["nc"]
    in_maps = _prep_inputs(
        inputs["x_BxTxH"], inputs["mask_BxT"], inputs["attention_bias_BxHxTxT"],
        inputs["Wq"], inputs["Wk"], inputs["Wv"], inputs["Wo"])
    res = run_bass_kernel_spmd(nc, in_maps, list(range(NCORES)), trace=trace)
    out = _gather(res.results)[None]       # restore batch dim [1, T, H]
    return out, res.exec_time_ns


def kernel(**inputs):
    out, _ = run(trace=False, **inputs)
    return out


# revision 30
# speedup vs baseline: 1.4523x; 1.0640x over previous
"""LocalSelfAttention (block-diagonal, block=50) Bass kernel for 8 trn2 cores.

Sharding: sequence-parallel over the 41 attention blocks (padded to 48 =
8 cores x 6 blocks). Each core computes QKV projections, block-local
multi-head attention, and the output projection for its 6 blocks (300
tokens, laid out as 6 x 64 partition-padded rows = 384). No collectives;
the host slices inputs per core and concatenates the per-core outputs.

All matmul data is fp16 (measured ~3e-4 per-matmul relative error on HW;
PSUM accumulation is fp32). Softmax runs without max-subtraction (logits
are O(+-6) for this problem family), with exp's accum_out providing the
denominator for free.

Scheduling notes (trn2):
- Engines execute their instruction streams in emission order, so the
  attention/softmax chain for head-group g is emitted right after the
  QT/KT projections of that head-group: its DVE/ACT work then overlaps
  the remaining projection matmuls on the PE.
- Matmuls whose lhsT partition bases differ (row groups 0 vs 64) can run
  concurrently inside the PE array and must not target the same PSUM
  bank (hardware collision) - AV outputs are split by block parity.
- Weight streaming uses [128, 1024] fp16 chunks alternating between the
  two HWDGE queues (sync + scalar) to keep DMA off the critical path.
"""

import sys
from contextlib import ExitStack

sys.path.insert(0, "/opt/trn_rl_repo")

import numpy as np

import concourse.bass as bass  # noqa: F401
import concourse.mybir as mybir
import concourse.tile as tile
from concourse import bacc
from concourse.bass_utils import run_bass_kernel_spmd

# ---- problem constants (hardcoded; kernel.py must be self-contained) ----
T, H = 2048, 2048
HEADS, DH = 16, 128
KBLK = 50          # attention block size (tokens)
NEG = -1e9
NCORES = 8
P = T + (KBLK - T % KBLK)          # padded seq len = 2050
NB = P // KBLK                     # 41 real blocks
NB_CORE = 6                        # blocks per core (8*6 = 48 >= 41)
BPAD = 64                          # per-block padded rows (64-aligned matmul bases)
TPAD = NB_CORE * BPAD              # 384 padded tokens per core
SCALE = DH ** -0.5
NH_T = H // 128                    # 16 h-tiles of 128
TPACK = NB_CORE * KBLK             # 300 packed tokens (QT/KT only)
F16 = mybir.dt.float16
F32 = mybir.dt.float32

_CACHE = {}


def _build_program():
    nc = bacc.Bacc("TRN2", target_bir_lowering=False, debug=False,
                   num_devices=NCORES)

    xT_d = nc.dram_tensor("xT", [H, TPAD], F16, kind="ExternalInput").ap()
    xTq_d = nc.dram_tensor("xTq", [H, TPACK], F16, kind="ExternalInput").ap()
    wq_d = nc.dram_tensor("wq", [H, H], F16, kind="ExternalInput").ap()
    wk_d = nc.dram_tensor("wk", [H, H], F16, kind="ExternalInput").ap()
    wv_d = nc.dram_tensor("wv", [H, H], F16, kind="ExternalInput").ap()
    wo_d = nc.dram_tensor("wo", [H, H], F16, kind="ExternalInput").ap()
    bias_d = nc.dram_tensor("bias", [KBLK, NB_CORE * HEADS * KBLK], F16,
                            kind="ExternalInput").ap()
    ident_d = nc.dram_tensor("ident", [128, 128], F16, kind="ExternalInput").ap()
    y_d = nc.dram_tensor("y", [TPAD, H], F32, kind="ExternalOutput").ap()

    with tile.TileContext(nc) as tc, ExitStack() as ctx:
        _emit_body(nc, tc, ctx, xT_d, xTq_d, wq_d, wk_d, wv_d, wo_d, bias_d,
                   ident_d, y_d)

    nc.compile()
    return nc


def _emit_body(nc, tc, ctx, xT_d, xTq_d, wq_d, wk_d, wv_d, wo_d, bias_d,
               ident_d, y_d):
    sb = ctx.enter_context(tc.tile_pool(name="persist", bufs=1))
    wpool = ctx.enter_context(tc.tile_pool(name="wpool", bufs=32))
    apool = ctx.enter_context(tc.tile_pool(name="apool", bufs=3))
    anpool = ctx.enter_context(tc.tile_pool(name="anpool", bufs=7))
    ps_proj = ctx.enter_context(tc.tile_pool(name="ps_proj", bufs=3, space="PSUM"))
    ps_s = ctx.enter_context(tc.tile_pool(name="ps_s", bufs=2, space="PSUM"))
    ps_at = ctx.enter_context(tc.tile_pool(name="ps_at", bufs=1, space="PSUM"))
    ps_ot = ctx.enter_context(tc.tile_pool(name="ps_ot", bufs=2, space="PSUM"))

    # ---- persistent SBUF arrays ----
    xT = [sb.tile([128, TPAD], F16, tag=f"xT{h}", name=f"xT{h}") for h in range(NH_T)]
    xTq = [sb.tile([128, TPACK], F16, tag=f"xTq{h}", name=f"xTq{h}")
           for h in range(NH_T)]
    qt = [sb.tile([128, TPACK], F16, tag=f"qt{e}", name=f"qt{e}") for e in range(HEADS)]
    kt = [sb.tile([128, TPACK], F16, tag=f"kt{e}", name=f"kt{e}") for e in range(HEADS)]
    ot = [sb.tile([128, TPAD], F16, tag=f"ot{e}", name=f"ot{e}") for e in range(HEADS)]
    vsb = [sb.tile([128, H], F16, tag=f"v{t}", name=f"vsb{t}") for t in range(3)]
    atb = [sb.tile([128, HEADS * KBLK], F16, tag=f"at{b}", name=f"atb{b}")
           for b in range(NB_CORE)]
    bias_sb = sb.tile([KBLK, NB_CORE * HEADS * KBLK], F16, tag="bias")
    ident = sb.tile([128, 128], F16, tag="ident")

    def wload(dst, src, h):
        # alternate the two HWDGE queues so weight streaming isn't serialized
        (nc.sync if h % 2 == 0 else nc.scalar).dma_start(dst, src)

    for e in range(HEADS):
        nc.vector.memset(ot[e][:], 0.0)  # OT gap columns are never written

    anrm_live = {}

    def attention_softmax(g):
        # S matmuls + softmax for heads 8g..8g+8 of every block -> anrm tiles.
        # Blocks are processed in pairs sharing [128, .] tiles at partition
        # bases 0/64, so transposes/reductions batch 2 blocks at once.
        # The additive bias is injected by an identity-matmul accumulation into
        # the S PSUM bank (start=True) BEFORE the S matmuls (start=False) - all
        # at row group 0, so no concurrent-row-group bank hazard. The softmax
        # scale rides on exp's free affine; bias was pre-scaled by sqrt(DH) on
        # the host.
        for bp in range(NB_CORE // 2):
            asb = apool.tile([128, 8 * BPAD], F16, tag="a_exp", name="asb")
            anrm = anpool.tile([128, 8 * BPAD], F16, tag="a_nrm", name="anrm")
            nc.vector.memset(asb[:], 0.0)   # gap rows feed the batched reduce
            nc.vector.memset(anrm[:], 0.0)  # transpose reads [114, .] spans
            anrm_live[(bp, g)] = anrm
            sums = apool.tile([128, 8], F32, tag="sums", name="sums")
            recip = apool.tile([128, 8], F32, tag="recip", name="recip")
            for par in range(2):
                b = 2 * bp + par
                pb = BPAD * par
                tcol = KBLK * b
                sp = ps_s.tile([KBLK, 8 * KBLK], F32, tag="s", name="sp")
                boff = (b * 2 + g) * 8 * KBLK
                nc.tensor.matmul(sp[:], ident[0:KBLK, 0:KBLK],
                                 bias_sb[:, boff:boff + 8 * KBLK],
                                 start=True, stop=False)
                for el in range(8):
                    e = 8 * g + el
                    nc.tensor.matmul(sp[:, KBLK * el:KBLK * (el + 1)],
                                     qt[e][:, tcol:tcol + KBLK],
                                     kt[e][:, tcol:tcol + KBLK],
                                     start=False, stop=(el == 7))
                nc.scalar.activation(
                    asb[pb:pb + KBLK, :].rearrange("p (e x) -> p e x", e=8)[:, :, 0:KBLK],
                    sp[:], mybir.ActivationFunctionType.Exp, scale=SCALE)
            nc.vector.reduce_sum(
                sums[:], asb.rearrange("p (e x) -> p e x", e=8)[:, :, 0:KBLK],
                axis=mybir.AxisListType.X)
            nc.vector.reciprocal(recip[:], sums[:])
            for par in range(2):
                pb = BPAD * par
                for el in range(8):
                    nc.vector.tensor_scalar_mul(
                        anrm[pb:pb + KBLK, BPAD * el:BPAD * el + KBLK],
                        asb[pb:pb + KBLK, BPAD * el:BPAD * el + KBLK],
                        recip[pb:pb + KBLK, el:el + 1])

    def attention_transpose(g):
        # emitted a phase later so the PE never stalls on the softmax chain;
        # one [114, 114] transpose covers 2 blocks x 2 heads
        for bp in range(NB_CORE // 2):
            anrm = anrm_live.pop((bp, g))
            for j in range(4):
                atp = ps_at.tile([128, 2 * BPAD - 14], F16, tag="atp", name="atp")
                nc.tensor.transpose(
                    atp[0:2 * BPAD - 14, :],
                    anrm[0:2 * BPAD - 14, 2 * BPAD * j:2 * BPAD * (j + 1) - 14],
                    ident[0:2 * BPAD - 14, 0:2 * BPAD - 14])
                for p_ in range(2):
                    e = 8 * g + 2 * j + p_
                    for par in range(2):
                        b = 2 * bp + par
                        base = BPAD * par
                        nc.vector.tensor_copy(
                            atb[b][base:base + KBLK, KBLK * e:KBLK * (e + 1)],
                            atp[BPAD * p_:BPAD * p_ + KBLK,
                                BPAD * par:BPAD * par + KBLK])

    # ---- QT/KT projections (per half of the heads) + interleaved attention --
    for g in range(2):
        for w_hbm, dst in ((wq_d, qt), (wk_d, kt)):
            wt = [wpool.tile([128, 1024], F16, tag="w", name="w")
                  for _ in range(NH_T)]
            for h in range(NH_T):
                wload(wt[h][:],
                      w_hbm[128 * h:128 * (h + 1), 1024 * g:1024 * (g + 1)], h)
                if g == 0 and w_hbm is wq_d:
                    # first pass also stages packed x, paired h-by-h with the
                    # weight chunks so the first QT matmuls start immediately
                    wload(xTq[h][:], xTq_d[128 * h:128 * (h + 1), :], h + 1)
                if g == 0 and w_hbm is wk_d:
                    # padded x is first needed by the V phase - keep it off the
                    # QT critical path
                    wload(xT[h][:], xT_d[128 * h:128 * (h + 1), :], h + 1)
            if g == 0 and w_hbm is wk_d:
                # bias/ident aren't needed until the first attention ops
                # (~35us in); keep them off the startup critical path
                nc.sync.dma_start(bias_sb[:], bias_d)
                nc.scalar.dma_start(ident[:], ident_d)
            for el in range(8):            # head within group
                e = 8 * g + el
                pt = ps_proj.tile([128, TPACK], F32, tag="proj", name="pt")
                for h in range(NH_T):
                    nc.tensor.matmul(pt[:], wt[h][:, 128 * el:128 * (el + 1)],
                                     xTq[h][:], start=(h == 0), stop=(h == NH_T - 1))
                nc.vector.tensor_copy(dst[e][:], pt[:])
        attention_softmax(g)
        if g == 1:
            attention_transpose(0)

    # ---- V projection, token-major: out[t, ed] = xT[h, t].T @ W[h, ed] ----
    wt_wo0 = None
    for g in range(2):
        if g == 1:
            attention_transpose(1)
        wt = [wpool.tile([128, 1024], F16, tag="w", name="w") for _ in range(NH_T)]
        for h in range(NH_T):
            wload(wt[h][:], wv_d[128 * h:128 * (h + 1), 1024 * g:1024 * (g + 1)], h)
        if g == 1:
            # stage Wo group 0 now; its slot-waits resolve as V-g0 chunks
            # release, well before the output projection needs the data
            wt_wo0 = [wpool.tile([128, 1024], F16, tag="w", name="w")
                      for _ in range(HEADS)]
            for e in range(HEADS):
                wload(wt_wo0[e][:], wo_d[128 * e:128 * (e + 1), 0:1024], e + 1)
        for eo in range(2):
            eg = 2 * g + eo
            for tt in range(3):            # token tiles of 128
                pt = ps_proj.tile([128, 512], F32, tag="proj", name="pt")
                for h in range(NH_T):
                    nc.tensor.matmul(pt[:], xT[h][:, 128 * tt:128 * (tt + 1)],
                                     wt[h][:, 512 * eo:512 * (eo + 1)],
                                     start=(h == 0), stop=(h == NH_T - 1))
                nc.scalar.copy(vsb[tt][:, 512 * eg:512 * (eg + 1)], pt[:])

    # ---- A^T @ V -> OT[dh, t] per head ----
    # Matmuls with different lhsT partition bases (row groups 0 vs 64) run
    # concurrently in the PE array and must not share a PSUM bank: use one
    # PSUM tile per block-parity, then strided copies into ot[e].
    for e in range(HEADS):
        opa = ps_ot.tile([128, TPAD], F32, tag="otp", name="opa")
        opb = ps_ot.tile([128, TPAD], F32, tag="otp", name="opb")
        opp = (opa, opb)
        for b in range(NB_CORE):
            par = b % 2
            base = BPAD * par
            nc.tensor.matmul(
                opp[par][:, 128 * (b // 2):128 * (b // 2) + KBLK],
                vsb[b // 2][base:base + KBLK, 128 * e:128 * (e + 1)],
                atb[b][base:base + KBLK, KBLK * e:KBLK * (e + 1)],
                start=True, stop=True)
        for par in range(2):
            src = opp[par].rearrange("p (b x) -> p b x", b=3)[:, :, 0:KBLK]
            dst = ot[e].rearrange("p (b q x) -> p b q x", b=3, q=2)[:, :, par, 0:KBLK]
            nc.scalar.copy(dst, src)

    # ---- output projection: y[t, hout] = OT[ed, t].T @ Wo[ed, hout] ----
    for g in range(2):
        if g == 0:
            wt = wt_wo0
        else:
            wt = [wpool.tile([128, 1024], F16, tag="w", name="w")
                  for _ in range(HEADS)]
            for e in range(HEADS):
                wload(wt[e][:], wo_d[128 * e:128 * (e + 1), 1024:2048], e)
        for ho in range(2):
            hg = 2 * g + ho
            for tt in range(3):
                pt = ps_proj.tile([128, 512], F32, tag="proj", name="pt")
                for e in range(HEADS):
                    nc.tensor.matmul(pt[:], ot[e][:, 128 * tt:128 * (tt + 1)],
                                     wt[e][:, 512 * ho:512 * (ho + 1)],
                                     start=(e == 0), stop=(e == HEADS - 1))
                ysb = apool.tile([128, 512], F32, tag="y", name="ysb")
                nc.vector.tensor_copy(ysb[:], pt[:])
                nc.sync.dma_start(
                    y_d[128 * tt:128 * (tt + 1), 512 * hg:512 * (hg + 1)], ysb[:])


def _prep_inputs(x, mask, bias, Wq, Wk, Wv, Wo):
    """Slice/pad/transpose the full inputs into per-core input maps."""
    x = np.asarray(x, np.float32).reshape(T, H)
    mask = np.asarray(mask, np.float32).reshape(T)
    bias = np.asarray(bias, np.float32).reshape(HEADS, T, T)

    # padded x (tokens) and mask, as in the reference
    xp = np.zeros((P, H), np.float32)
    xp[:T] = x
    mb = np.full(P, NEG, np.float32)
    mb[:T] = mask

    # combined per-block additive bias: block-diag of attention bias + pair mask
    comb = np.zeros((NB_CORE * NCORES, HEADS, KBLK, KBLK), np.float32)
    for b in range(NB):
        lo, hi = KBLK * b, KBLK * (b + 1)
        blk = np.zeros((HEADS, KBLK, KBLK), np.float32)
        lim = min(hi, T) - lo
        if lim > 0:
            blk[:, :lim, :lim] = bias[:, lo:lo + lim, lo:lo + lim]
        pair = mb[lo:hi, None] * mb[None, lo:hi]
        # pre-scaled by sqrt(DH) (exp applies scale=DH**-0.5 to S + bias);
        # masked entries use -4000 (fp16-safe; * SCALE -> -354, exp -> 0)
        blk = blk * (DH ** 0.5) + np.where(pair > 0, 0.0, -4000.0)[None]
        comb[b] = blk

    wq = np.ascontiguousarray(Wq.reshape(H, H), np.float32).astype(np.float16)
    wk = np.ascontiguousarray(Wk.reshape(H, H), np.float32).astype(np.float16)
    wv = np.ascontiguousarray(Wv.reshape(H, H), np.float32).astype(np.float16)
    wo = np.ascontiguousarray(Wo.reshape(H, H), np.float32).astype(np.float16)
    ident = np.eye(128, dtype=np.float16)

    in_maps = []
    for c in range(NCORES):
        xc = np.zeros((TPAD, H), np.float32)
        for j in range(NB_CORE):
            b = NB_CORE * c + j
            if b < NB:
                xc[BPAD * j:BPAD * j + KBLK] = xp[KBLK * b:KBLK * (b + 1)]
        # bias layout: [q, (block, headgroup, head, k)], fp16
        cc = comb[NB_CORE * c:NB_CORE * (c + 1)]          # [6, 16, 50, 50]
        bc = cc.transpose(2, 0, 1, 3).reshape(KBLK, NB_CORE * HEADS * KBLK)
        bc = bc.astype(np.float16)
        xq = np.zeros((TPACK, H), np.float32)
        for j in range(NB_CORE):
            b = NB_CORE * c + j
            if b < NB:
                xq[KBLK * j:KBLK * (j + 1)] = xp[KBLK * b:KBLK * (b + 1)]
        in_maps.append({
            "xT": np.ascontiguousarray(xc.T).astype(np.float16),
            "xTq": np.ascontiguousarray(xq.T).astype(np.float16),
            "wq": wq, "wk": wk, "wv": wv, "wo": wo,
            "bias": np.ascontiguousarray(bc),
            "ident": ident,
        })
    return in_maps


def _gather(results):
    out = np.empty((T, H), np.float32)
    for c in range(NCORES):
        yc = results[c]["y"]
        for j in range(NB_CORE):
            b = NB_CORE * c + j
            if b >= NB:
                continue
            n = min(KBLK * (b + 1), T) - KBLK * b
            if n > 0:
                out[KBLK * b:KBLK * b + n] = yc[BPAD * j:BPAD * j + n]
    return out


def run(trace=False, **inputs):
    if "nc" not in _CACHE:
        _CACHE["nc"] = _build_program()
    nc = _CACHE["nc"]
    in_maps = _prep_inputs(
        inputs["x_BxTxH"], inputs["mask_BxT"], inputs["attention_bias_BxHxTxT"],
        inputs["Wq"], inputs["Wk"], inputs["Wv"], inputs["Wo"])
    res = run_bass_kernel_spmd(nc, in_maps, list(range(NCORES)), trace=trace)
    out = _gather(res.results)[None]       # restore batch dim [1, T, H]
    return out, res.exec_time_ns


def kernel(**inputs):
    out, _ = run(trace=False, **inputs)
    return out
